# revision 5
# baseline (speedup 1.0000x reference)
"""AttentionLSTM Trainium2 kernel v2: 8-core DATA-parallel (16 samples/core),
zero per-step collectives, recurrent body in a For_i hardware loop.

Cost model for this (axon/fake_nrt) stack, measured by microbenchmark:
  - per STATIC instruction per call: ~24 us  -> keep the NEFF tiny (For_i)
  - per collective: ~400 us                  -> only 3 prologue AllGathers
  - dynamic loop iterations: ~2 us/instr serialized, less when pipelined

Design:
  - core k owns samples n in [16k, 16k+16). Weights are sharded on the host
    (row-chunks), AllGathered once on device, held in SBUF as fp16.
  - prologue GEMM: xw = x@Wx + b for all T in a For_i over 8 row-tiles.
  - per step (For_i over t): attention in an (n,m) partition layout
    (p = n*8+m owns A[n, m*128:(m+1)*128, :]), selector matmuls for the
    cross-chunk score sum and softmax broadcast, PE transposes to build the
    fp16 lhsT tiles of h and attn, 136 PSUM-accumulating matmuls for
    a = xw_t + h@Wh + attn@(32*Wattn), fused sigmoid/tanh, cell update.
  - state layout [32, 512]: partition q*16+n holds h[n, q*512:(q+1)*512].
  - output written per step as [32, 512] -> out_d[:, t*512:...]; host fixes
    the layout.
"""

import os
import sys

sys.path.insert(0, "/opt/trn_rl_repo")

import numpy as np

import concourse.bass as bass
import concourse.tile as tile
from concourse import bacc, mybir
from concourse.bass import ds, ts
from concourse.bass_utils import run_bass_kernel_spmd

N, T, D, H = 128, 64, 1024, 1024
L = 16
NC = 8
NS = N // NC           # samples per core = 16
P = 128
G = 4 * H              # gate cols = 4096
KC = 8                 # 128-row contraction chunks in D/H

F32 = mybir.dt.float32
F32R = mybir.dt.float32r
F16 = mybir.dt.float16

_cache = {}


def _build(t_steps: int, use_cc: bool = True, repeat: int = 1, probe: str = ""):
    nc = bacc.Bacc(
        "TRN2",
        target_bir_lowering=False,
        debug=False,
        enable_asserts=False,
        num_devices=NC,
    )

    # ---- kernel I/O ----
    xs = nc.dram_tensor("xs", [P, KC * NS * T], F16, kind="ExternalInput")
    wxs = nc.dram_tensor("wxs", [P, G], F16, kind="ExternalInput")
    whs = nc.dram_tensor("whs", [P, G], F16, kind="ExternalInput")
    was = nc.dram_tensor("was", [P, G], F16, kind="ExternalInput")
    bia = nc.dram_tensor("bia", [1, G], F16, kind="ExternalInput")
    ascs = nc.dram_tensor("ascs", [P, L * P], F16, kind="ExternalInput")
    c0s = nc.dram_tensor("c0s", [NS, H], F16, kind="ExternalInput")
    selM = nc.dram_tensor("selM", [P, NS], F16, kind="ExternalInput")
    selMT = nc.dram_tensor("selMT", [NS, P], F16, kind="ExternalInput")
    eyeT = nc.dram_tensor("eyeT", [P, P], F32R, kind="ExternalInput")
    eye16 = nc.dram_tensor("eye16", [NS, NS], F16, kind="ExternalInput")
    ones1 = nc.dram_tensor("ones1", [1, P], F16, kind="ExternalInput")
    out = nc.dram_tensor("out", [NS, T * H], F16, kind="ExternalOutput")

    # ---- internal DRAM ----
    wxi = nc.dram_tensor("wxi", [P, G], F16)
    whi = nc.dram_tensor("whi", [P, G], F16)
    wai = nc.dram_tensor("wai", [P, G], F16)
    wx_g = nc.dram_tensor("wx_g", [D, G], F16, addr_space="Shared")
    wh_g = nc.dram_tensor("wh_g", [H, G], F16, addr_space="Shared")
    wa_g = nc.dram_tensor("wa_g", [H, G], F16, addr_space="Shared")
    xw_dram = nc.dram_tensor("xw_dram", [NS * T, G], F16)
    h_d = nc.dram_tensor("h_d", [NS, H], F32R)

    rg = [list(range(NC))]

    with tile.TileContext(nc) as tc:
        # ---- one-time weight AllGathers (rank k supplies rows 128k..) ----
        nc.sync.dma_start(wxi[:, :], wxs[:, :])
        nc.sync.dma_start(whi[:, :], whs[:, :])
        nc.sync.dma_start(wai[:, :], was[:, :])
        if use_cc:
            nc.gpsimd.collective_compute(
                "AllGather", mybir.AluOpType.bypass, replica_groups=rg,
                ins=[wxi.ap()], outs=[wx_g.ap()])
            nc.gpsimd.collective_compute(
                "AllGather", mybir.AluOpType.bypass, replica_groups=rg,
                ins=[whi.ap()], outs=[wh_g.ap()])
            nc.gpsimd.collective_compute(
                "AllGather", mybir.AluOpType.bypass, replica_groups=rg,
                ins=[wai.ap()], outs=[wa_g.ap()])
        else:
            for m in range(KC):
                nc.sync.dma_start(wx_g[m * P:(m + 1) * P, :], wxi[:, :])
                nc.sync.dma_start(wh_g[m * P:(m + 1) * P, :], whi[:, :])
                nc.sync.dma_start(wa_g[m * P:(m + 1) * P, :], wai[:, :])

        with tc.tile_pool(name="static", bufs=1) as sp, \
             tc.tile_pool(name="state", bufs=1) as statep:
            # ---- persistent SBUF ----
            wh_sb = []
            wa_sb = []
            for m in range(KC):
                t_ = sp.tile([P, G], F16, tag=f"wh{m}")
                nc.sync.dma_start(t_[:], wh_g[m * P:(m + 1) * P, :])
                wh_sb.append(t_)
            for m in range(KC):
                t_ = sp.tile([P, G], F16, tag=f"wa{m}")
                nc.sync.dma_start(t_[:], wa_g[m * P:(m + 1) * P, :])
                wa_sb.append(t_)
            asc_sb = sp.tile([P, L * P], F32, tag="asc")
            eye = sp.tile([P, P], F32R, tag="eye")
            nc.sync.dma_start(eye[:], eyeT[:, :])
            e16 = sp.tile([NS, NS], F16, tag="e16")
            nc.sync.dma_start(e16[:], eye16[:, :])
            sM = sp.tile([P, NS], F16, tag="sM")
            nc.sync.dma_start(sM[:], selM[:, :])
            sMT = sp.tile([NS, P], F16, tag="sMT")
            nc.sync.dma_start(sMT[:], selMT[:, :])
            on1 = sp.tile([1, P], F16, tag="on1")
            nc.sync.dma_start(on1[:], ones1[:, :])
            bia_sb = sp.tile([1, G], F16, tag="bia")
            nc.sync.dma_start(bia_sb[:], bia[:, :])

            c_st = statep.tile([NS, H], F32, tag="c")
            h_st = statep.tile([NS, H], F32R, tag="h")
            c016 = statep.tile([NS, H], F16, tag="c016")
            nc.sync.dma_start(c016[:], c0s[:, :])
            nc.vector.tensor_copy(c_st[:], c016[:])
            nc.vector.tensor_copy(h_st[:], c_st[:])
            nc.sync.dma_start(h_d[:, :], h_st[:])

            # ============ prologue: xw = x@Wx + b ============
            with tc.tile_pool(name="xsp", bufs=2) as xsp, \
                 tc.tile_pool(name="wxp", bufs=2) as wxp, \
                 tc.tile_pool(name="xwo", bufs=2) as xwop, \
                 tc.tile_pool(name="xwps", bufs=1, space="PSUM") as xwps:
                asc16 = xsp.tile([P, L * P], F16, tag="asc16")
                nc.sync.dma_start(asc16[:], ascs[:, :])
                nc.vector.tensor_copy(asc_sb[:], asc16[:])
                xw_ps = xwps.tile([P, G], F32, tag="xwps")
                with tc.For_i(0, KC, 1) as rt:
                    # x-block for this row-tile: [128 d, (m), 128 r]
                    xsc = xsp.tile([P, KC * P], F16, tag="xsc")
                    nc.sync.dma_start(
                        xsc[:].rearrange("p (m r) -> p m r", m=KC),
                        xs.rearrange("p (m r) -> p m r", m=KC)
                        [:, :, ds(rt * P, P)])
                    for gt in range(KC):
                        nc.tensor.matmul(
                            xw_ps[:, gt * 512:(gt + 1) * 512], on1[:],
                            bia_sb[:, gt * 512:(gt + 1) * 512],
                            start=True, stop=False)
                    for m in range(KC):
                        wxc = wxp.tile([P, G], F16, tag="wxc")
                        nc.sync.dma_start(wxc[:], wx_g[m * P:(m + 1) * P, :])
                        for gt in range(KC):
                            nc.tensor.matmul(
                                xw_ps[:, gt * 512:(gt + 1) * 512],
                                xsc[:, m * P:(m + 1) * P],
                                wxc[:, gt * 512:(gt + 1) * 512],
                                start=False, stop=(m == KC - 1))
                    xw_o = xwop.tile([P, G], F16, tag="xwo")
                    nc.vector.tensor_copy(xw_o[:], xw_ps[:])
                    nc.sync.dma_start(xw_dram[ds(rt * P, P), :], xw_o[:])

            # ============ recurrent loop ============
            with tc.tile_pool(name="hTp", bufs=1) as hTp, \
                 tc.tile_pool(name="nmp", bufs=1) as nmp, \
                 tc.tile_pool(name="smp", bufs=1) as smp, \
                 tc.tile_pool(name="xwsb", bufs=1) as xwsb, \
                 tc.tile_pool(name="gp", bufs=1) as gp, \
                 tc.tile_pool(name="aps", bufs=1, space="PSUM") as apsp, \
                 tc.tile_pool(name="tps", bufs=1, space="PSUM") as tpsp:

                n_mm = 0 if "nomm" in probe else KC

                def half_gates_xh(a_ps, xw_t, hT, ph):
                    # xw-init + h@Wh part: depends only on hT, so the PE can
                    # run it while the DVE attention chain produces atT
                    for g2 in range(4):
                        gt = ph * 4 + g2
                        o_ = a_ps[:, g2 * 512:(g2 + 1) * 512]
                        nc.tensor.matmul(
                            o_, e16[:], xw_t[:, gt * 512:(gt + 1) * 512],
                            start=True, stop=(n_mm == 0))
                        for m in range(n_mm):
                            nc.tensor.matmul(
                                o_, hT[:, m * NS:(m + 1) * NS],
                                wh_sb[m][:, gt * 512:(gt + 1) * 512],
                                start=False, stop=False)

                def half_gates_at(a_ps, atTr, ph):
                    # (attn/32)@(32*Wattn) part closes each accumulation
                    for g2 in range(4):
                        gt = ph * 4 + g2
                        o_ = a_ps[:, g2 * 512:(g2 + 1) * 512]
                        for m in range(n_mm):
                            nc.tensor.matmul(
                                o_, atTr[:, m, :],
                                wa_sb[m][:, gt * 512:(gt + 1) * 512],
                                start=False, stop=(m == KC - 1))

                def body(i):
                    # -- xw prefetch (depends only on i)
                    xw_t = xwsb.tile([NS, G], F16, tag="xw")
                    if "nodma" not in probe:
                        nc.sync.dma_start(xw_t[:], xw_dram[ds(i * NS, NS), :])
                    else:
                        nc.any.memzero(xw_t)

                    # -- hT: 8 transposes [16,128] -> [128,16] into one bank
                    hT_ps = tpsp.tile([P, P], F32R, tag="hT")
                    if "notr" in probe:
                        nc.any.memzero(hT_ps)
                    for m in range(0 if "notr" in probe else KC):
                        nc.tensor.transpose(
                            hT_ps[:, m * NS:(m + 1) * NS],
                            h_st[:, m * P:(m + 1) * P],
                            eye[0:NS, 0:NS])
                    hT = hTp.tile([P, P], F16, tag="hTf")
                    nc.vector.tensor_copy(hT[:], hT_ps[:])
                    a_ps = apsp.tile([NS, 2048], F32, tag="a")
                    half_gates_xh(a_ps, xw_t, hT, 0)

                    # -- h_nm [128 (n,m), 128 hc] from the h_d DRAM copy
                    h_nm = nmp.tile([P, P], F32R, tag="hnm")
                    if "nodma" not in probe:
                        nc.sync.dma_start(
                            h_nm[:],
                            h_d.rearrange("n (m c) -> (n m) c", m=KC))
                    else:
                        nc.any.memzero(h_nm)

                    # -- scores: per-partition partial over 128 h-cols
                    novec = "novec" in probe
                    prod = smp.tile([P, L * P], F32, tag="prod")
                    if not novec:
                        nc.vector.tensor_tensor(
                        prod[:],
                        h_nm[:].unsqueeze(1).broadcast_to((P, L, P)),
                        asc_sb[:].rearrange("p (l c) -> p l c", l=L),
                            op=mybir.AluOpType.mult)
                    sp_ = smp.tile([P, L], F16, tag="sp")
                    if not novec:
                        with nc.allow_low_precision(
                                reason="DVE ALU accumulates fp32; f16 store"):
                            nc.vector.tensor_reduce(
                                sp_[:],
                                prod[:].rearrange("p (l c) -> p l c", l=L),
                                axis=mybir.AxisListType.X,
                                op=mybir.AluOpType.add)
                    else:
                        nc.vector.tensor_copy(
                            sp_[:], asc_sb[:, 0:L].bitcast(F32))
                    # -- cross-chunk sum: sc[n, l] = sum_m sp[(n,m), l]
                    sc_ps = tpsp.tile([NS, L], F32, tag="sc")
                    nc.tensor.matmul(sc_ps[:], sM[:], sp_[:],
                                     start=True, stop=True)
                    # -- softmax over l (no max-subtract: |scores| <= 32)
                    ex = smp.tile([NS, L], F32, tag="ex")
                    ssum = smp.tile([NS, 1], F32, tag="ssum")
                    nc.scalar.activation(
                        ex[:], sc_ps[:], mybir.ActivationFunctionType.Exp,
                        accum_out=ssum[:])
                    rcp = smp.tile([NS, 1], F32, tag="rcp")
                    nc.vector.reciprocal(rcp[:], ssum[:])
                    wgt = smp.tile([NS, L], F16, tag="wgt")
                    nc.vector.tensor_scalar_mul(wgt[:], ex[:], rcp[:])
                    # -- broadcast w back to (n,m) partitions
                    wb_ps = tpsp.tile([P, L], F32, tag="wb")
                    nc.tensor.matmul(wb_ps[:], sMT[:], wgt[:],
                                     start=True, stop=True)

                    # -- attn_nm[(n,m), hc] = sum_l asc * w  (= attn/32)
                    prod2 = smp.tile([P, L * P], F32, tag="prod")
                    attn_nm = nmp.tile([P, P], F32R, tag="attn")
                    if not novec:
                        nc.vector.tensor_tensor(
                            prod2[:],
                            wb_ps[:].unsqueeze(2).broadcast_to((P, L, P)),
                            asc_sb[:].rearrange("p (l c) -> p l c", l=L),
                            op=mybir.AluOpType.mult)
                        with nc.allow_low_precision(reason="f32r bits f32"):
                            nc.vector.tensor_reduce(
                                attn_nm[:],
                                prod2[:].rearrange("p (l c) -> p c l", l=L),
                                axis=mybir.AxisListType.X,
                                op=mybir.AluOpType.add)
                    else:
                        nc.vector.tensor_copy(
                            attn_nm[:], h_nm[:])
                    # -- atT[hc, (n,m)]: one [128,128] transpose
                    atT_ps = tpsp.tile([P, P], F32R, tag="atT")
                    if "notr" in probe:
                        nc.any.memzero(atT_ps)
                    else:
                        nc.tensor.transpose(atT_ps[:], attn_nm[:], eye[:])
                    atT = hTp.tile([P, P], F16, tag="atTf")
                    nc.vector.tensor_copy(atT[:], atT_ps[:])
                    atTr = atT[:].rearrange("p (n m) -> p m n", m=KC)

                    # -- phase 1: gates i and tanh-gate g (permuted cols)
                    half_gates_at(a_ps, atTr, 0)
                    sig_i = gp.tile([NS, H], F32, tag="sig_i")
                    nc.scalar.activation(sig_i[:], a_ps[:, 0:H],
                                         mybir.ActivationFunctionType.Sigmoid)
                    tg = gp.tile([NS, H], F32, tag="tg")
                    nc.scalar.activation(tg[:], a_ps[:, H:2 * H],
                                         mybir.ActivationFunctionType.Tanh)
                    ig = gp.tile([NS, H], F32, tag="ig")
                    nc.vector.tensor_mul(ig[:], sig_i[:], tg[:])

                    # -- phase 2: gates f and o (same psum buffer, rotated)
                    a_ps2 = apsp.tile([NS, 2048], F32, tag="a")
                    half_gates_xh(a_ps2, xw_t, hT, 1)
                    half_gates_at(a_ps2, atTr, 1)
                    sig_fo = gp.tile([NS, 2 * H], F32, tag="sig_fo")
                    nc.scalar.activation(sig_fo[:], a_ps2[:],
                                         mybir.ActivationFunctionType.Sigmoid)
                    fc = gp.tile([NS, H], F32, tag="fc")
                    nc.vector.tensor_mul(fc[:], sig_fo[:, 0:H], c_st[:])
                    nc.vector.tensor_add(c_st[:], fc[:], ig[:])
                    th = gp.tile([NS, H], F32, tag="th")
                    nc.scalar.activation(th[:], c_st[:],
                                         mybir.ActivationFunctionType.Tanh)
                    nc.vector.tensor_mul(h_st[:], sig_fo[:, H:2 * H], th[:])

                    h16 = gp.tile([NS, H], F16, tag="h16")
                    nc.vector.tensor_copy(h16[:], h_st[:])
                    if "nodma" not in probe:
                        nc.sync.dma_start(out[:, ds(i * H, H)], h16[:])
                        nc.sync.dma_start(h_d[:, :], h_st[:])

                if repeat == 1:
                    with tc.For_i(0, t_steps, 1) as i:
                        body(i)
                else:
                    with tc.For_i(0, repeat, 1) as r_i:
                        with tc.For_i(0, t_steps, 1) as i:
                            body(i)

    nc.compile()
    return nc


def _prep_weights(Wx, Wh, Wattn, b):
    Wx = np.asarray(Wx, np.float32)
    Wh = np.asarray(Wh, np.float32)
    Wattn = np.asarray(Wattn, np.float32)
    b = np.asarray(b, np.float32)
    selM = np.zeros((P, NS), dtype=np.float16)
    for n in range(NS):
        selM[n * KC:(n + 1) * KC, n] = 1.0
    # gate-column permutation: [i, g, f, o] so phase1=(i,g), phase2=(f,o)
    perm = np.concatenate([np.arange(0, H), np.arange(3 * H, 4 * H),
                           np.arange(H, 2 * H), np.arange(2 * H, 3 * H)])
    return {
        "wx": Wx[:, perm].astype(np.float16),
        "wh": Wh[:, perm].astype(np.float16),
        "wa": (Wattn[:, perm] * 32.0).astype(np.float16),
        "bia": b[perm].reshape(1, G).astype(np.float16),
        "selM": selM,
        "selMT": np.ascontiguousarray(selM.T),
        "eye": np.eye(P, dtype=np.float32),
        "e16": np.eye(NS, dtype=np.float16),
        "on1": np.ones((1, P), dtype=np.float16),
    }


def _prep_inputs(x, A, wdict):
    x = np.asarray(x, np.float32)
    A = np.asarray(A, np.float32)
    A_flat = A.reshape(N, H, L)
    c0 = A.mean(axis=(2, 3)).astype(np.float32)  # [N, H]

    in_maps = []
    for k in range(NC):
        sl = slice(k * NS, (k + 1) * NS)
        # xs[p, m*NS*T + t*NS + n] = x[nk+n, t, m*128+p]
        xsl = x[sl].transpose(2, 1, 0).astype(np.float16)   # [D, T, NS]
        xsl = xsl.reshape(KC, P, T * NS).transpose(1, 0, 2).reshape(
            P, KC * T * NS)
        # ascs[n*8+m, l*128+hc] = A[nk+n, m*128+hc, l] / 32
        asl = A_flat[sl].reshape(NS, KC, P, L).transpose(0, 1, 3, 2)
        asl = (asl / 32.0).reshape(P, L * P).astype(np.float16)
        in_maps.append({
            "c0s": np.ascontiguousarray(c0[sl]).astype(np.float16),
            "xs": np.ascontiguousarray(xsl),
            "ascs": np.ascontiguousarray(asl),
            "wxs": np.ascontiguousarray(wdict["wx"][k * P:(k + 1) * P, :]),
            "whs": np.ascontiguousarray(wdict["wh"][k * P:(k + 1) * P, :]),
            "was": np.ascontiguousarray(wdict["wa"][k * P:(k + 1) * P, :]),
            "bia": wdict["bia"],
            "selM": wdict["selM"],
            "selMT": wdict["selMT"],
            "eyeT": wdict["eye"],
            "eye16": wdict["e16"],
            "ones1": wdict["on1"],
        })
    return in_maps


_prep_cache = {}


def _prep_key(*arrs):
    parts = []
    for a in arrs:
        a = np.asarray(a)
        flat = a.reshape(-1)
        stride = max(1, flat.size // 2048)
        parts.append((a.shape, str(a.dtype), flat[::stride].tobytes()))
    return hash(tuple(parts))


def kernel(x, A, Wx, Wh, Wattn, b, t_steps=T, use_cc=True, repeat=1,
           probe=""):
    key = (t_steps, use_cc, repeat, probe)
    if key not in _cache:
        _cache[key] = _build(t_steps, use_cc, repeat, probe)
    nc = _cache[key]
    pkey = _prep_key(x, A, Wx, Wh, Wattn, b)
    if pkey not in _prep_cache:
        _prep_cache.clear()
        _prep_cache[pkey] = _prep_inputs(x, A, _prep_weights(Wx, Wh, Wattn, b))
    in_maps = _prep_cache[pkey]
    res = run_bass_kernel_spmd(nc, in_maps, core_ids=list(range(NC)),
                               trace=False)
    global LAST_EXEC_NS
    LAST_EXEC_NS = res.exec_time_ns
    outp = np.empty((N, t_steps, H), np.float32)
    for k in range(NC):
        o = res.results[k]["out"].reshape(NS, T, H)
        outp[k * NS:(k + 1) * NS] = o[:, :t_steps, :].astype(np.float32)
    return outp


LAST_EXEC_NS = None
